# revision 4
# baseline (speedup 1.0000x reference)
"""CrossModalityAttention Trainium2 kernel (8 NeuronCores, SPMD).

Sharding: core c -> batch b = c//4, head-group hg = c%4 (4 of 16 heads).
Each core computes LN + QKV projections for its heads, full cross-attention
(self K/V concat context K/V), and a partial output projection; partials are
ReduceScattered over the 4 cores of each batch (core gets row-quarter hg),
residual (+ b_out) added, and the host reassembles the [2, 2048, 1024] output.

Precision: LN stats/apply and softmax denominators in fp32; all matmul
operands bf16 with fp32 PSUM accumulation. Softmax skips max-subtraction
(logits are O(3) for this problem family) so only exp + ones-row-matmul
normalization is needed.
"""
import sys
import numpy as np
import ml_dtypes

for p in ("/root/.axon_site", "/root/.axon_site/_ro/trn_rl_repo",
          "/root/.axon_site/_ro/pypackages", "/opt/trn_rl_repo"):
    if p not in sys.path:
        sys.path.append(p)

import concourse.bass as bass
from concourse import bacc
import concourse.mybir as mybir
import concourse.tile as tile
from concourse.bass_utils import run_bass_kernel_spmd

f32 = mybir.dt.float32
bf16 = mybir.dt.bfloat16

B, T, S, DIM = 2, 2048, 2048, 1024
HEADS, HEAD_DIM = 16, 64
HPC = 4                   # heads per core
HCOLS = HPC * HEAD_DIM    # 256 channel columns per core
N_CORES = 8
CORE_IDS = list(range(N_CORES))
EPS = 1e-5

NT = T // 128             # 16 t-tiles per batch
NCHUNK = 4                # t-chunks of 512
NSB = (T + S) // 128      # 32 s-blocks of the concat sequence
VW = HEAD_DIM + 1         # V columns + ones column per head


def _build(trace_label=""):
    nc = bacc.Bacc("TRN2", target_bir_lowering=False, debug=False,
                   num_devices=N_CORES)

    XB = nc.dram_tensor("xb", [T, DIM], f32, kind="ExternalInput").ap()
    CB = nc.dram_tensor("cb", [S, DIM], f32, kind="ExternalInput").ap()
    WQ = nc.dram_tensor("wq", [DIM, HCOLS], bf16, kind="ExternalInput").ap()
    WK = nc.dram_tensor("wk", [DIM, HCOLS], bf16, kind="ExternalInput").ap()
    WV = nc.dram_tensor("wv", [DIM, HCOLS], bf16, kind="ExternalInput").ap()
    WO = nc.dram_tensor("wo", [HCOLS, DIM], bf16, kind="ExternalInput").ap()
    BQ = nc.dram_tensor("bq", [HCOLS], f32, kind="ExternalInput").ap()
    BK = nc.dram_tensor("bk", [HCOLS], f32, kind="ExternalInput").ap()
    BV = nc.dram_tensor("bv", [HCOLS], f32, kind="ExternalInput").ap()
    RES = nc.dram_tensor("res", [T // 4, DIM], f32, kind="ExternalInput").ap()
    IDN = nc.dram_tensor("idn", [128, 128], bf16, kind="ExternalInput").ap()

    OUT = nc.dram_tensor("out", [T // 4, DIM], f32, kind="ExternalOutput").ap()

    partial = nc.dram_tensor("partial", [T, DIM], f32).ap()
    rs_out = nc.dram_tensor("rs_out", [T // 4, DIM], f32).ap()
    zscr = nc.dram_tensor("zscr", [16, 512], f32).ap()

    with tile.TileContext(nc) as tc:
        with (
            tc.tile_pool(name="persist", bufs=1) as per,
            tc.tile_pool(name="stream", bufs=3) as st,
            tc.tile_pool(name="xnp", bufs=10) as xnp,
            tc.tile_pool(name="xntp", bufs=2) as xntp,
            tc.tile_pool(name="ep", bufs=4) as ep,
            tc.tile_pool(name="zp", bufs=4) as zp,
        ):
            # ---------------- persistent tiles ----------------
            wq_sb = per.tile([128, 8, HCOLS], bf16, tag="wq")
            wk_sb = per.tile([128, 8, HCOLS], bf16, tag="wk")
            wv_sb = per.tile([128, 8, HCOLS], bf16, tag="wv")
            wo_sb = per.tile([128, 2, DIM], bf16, tag="wo")
            nc.sync.dma_start(out=wq_sb, in_=WQ.rearrange("(a p) c -> p a c", p=128))
            nc.sync.dma_start(out=wk_sb, in_=WK.rearrange("(a p) c -> p a c", p=128))
            nc.sync.dma_start(out=wv_sb, in_=WV.rearrange("(a p) c -> p a c", p=128))
            nc.sync.dma_start(out=wo_sb, in_=WO.rearrange("(a p) c -> p a c", p=128))

            bq_sb = per.tile([128, 2], f32, tag="bq")
            bk_sb = per.tile([128, 2], f32, tag="bk")
            nc.sync.dma_start(out=bq_sb, in_=BQ.rearrange("(a p) -> p a", p=128))
            nc.sync.dma_start(out=bk_sb, in_=BK.rearrange("(a p) -> p a", p=128))
            bvb = per.tile([128, HCOLS], f32, tag="bvb")
            nc.sync.dma_start(out=bvb, in_=bass.AP(
                tensor=BV.tensor, offset=0, ap=[[0, 128], [1, HCOLS]]))

            ident = per.tile([128, 128], bf16, tag="ident")
            nc.sync.dma_start(out=ident, in_=IDN)
            eps_sb = per.tile([128, 1], f32, tag="eps")
            nc.vector.memset(eps_sb, EPS)

            qt_sb = per.tile([128, 2, T], bf16, tag="qt")      # Q^T
            kt_sb = per.tile([128, 2, T + S], bf16, tag="kt")  # K^T (concat)
            v_sb = per.tile([128, NSB, HPC * VW], bf16, tag="v")   # V | ones
            aot_sb = per.tile([128, 2, T], bf16, tag="aot")    # attn out^T

            for h in range(HPC):  # ones columns for Z rows
                nc.vector.memset(v_sb[:, :, h * VW + HEAD_DIM: (h + 1) * VW], 1.0)

            # ---------------- phase A: LN + transposes + QKV ----------------
            with tc.tile_pool(name="psA", bufs=2, space="PSUM") as psA:
                for src_i, SRC in ((0, XB), (1, CB)):
                    for ch in range(NCHUNK):
                        xn_tiles = []
                        for tt in range(4):
                            r0 = (ch * 4 + tt) * 128
                            xt = st.tile([128, DIM], f32, tag="xt")
                            nc.sync.dma_start(out=xt, in_=SRC[r0:r0 + 128, :])
                            # LN stats
                            stats = st.tile([128, 2, 6], f32, tag="stats")
                            xr = xt[:].rearrange("p (a q) -> p a q", a=2)
                            for sg in range(2):
                                nc.vector.bn_stats(out=stats[:, sg, :], in_=xr[:, sg, :])
                            mv = st.tile([128, 2], f32, tag="mv")
                            nc.vector.bn_aggr(out=mv, in_=stats)
                            rstd = st.tile([128, 1], f32, tag="rstd")
                            nc.scalar.activation(
                                out=rstd, in_=mv[:, 1:2],
                                func=mybir.ActivationFunctionType.Sqrt,
                                bias=eps_sb, scale=1.0)
                            nc.vector.reciprocal(out=rstd, in_=rstd)
                            xn = xnp.tile([128, DIM], bf16, tag="xn")
                            nc.vector.tensor_scalar(
                                out=xn, in0=xt, scalar1=mv[:, 0:1], scalar2=rstd,
                                op0=mybir.AluOpType.subtract,
                                op1=mybir.AluOpType.mult)
                            xn_tiles.append(xn)

                        # transpose chunk -> xnT [128c, 8ckt, 512t]
                        xnt = xntp.tile([128, 8, 512], bf16, tag="xnt")
                        for ckt in range(8):
                            pt = psA.tile([128, 512], bf16, tag="tp")
                            for tt in range(4):
                                nc.tensor.transpose(
                                    pt[:, tt * 128:(tt + 1) * 128],
                                    xn_tiles[tt][:, ckt * 128:(ckt + 1) * 128],
                                    ident)
                            nc.vector.tensor_copy(xnt[:, ckt, :], pt)

                        # Q^T / K^T projections for this chunk
                        wlist = ([(wq_sb, bq_sb, qt_sb, 0), (wk_sb, bk_sb, kt_sb, 0)]
                                 if src_i == 0 else [(wk_sb, bk_sb, kt_sb, T)])
                        for (w, bia, dst, off) in wlist:
                            for kt_o in range(2):
                                pq = psA.tile([128, 512], f32, tag="proj")
                                for ckt in range(8):
                                    nc.tensor.matmul(
                                        pq,
                                        lhsT=w[:, ckt, kt_o * 128:(kt_o + 1) * 128],
                                        rhs=xnt[:, ckt, :],
                                        start=(ckt == 0), stop=(ckt == 7))
                                nc.vector.tensor_scalar(
                                    out=dst[:, kt_o, off + ch * 512: off + (ch + 1) * 512],
                                    in0=pq, scalar1=bia[:, kt_o:kt_o + 1],
                                    scalar2=None, op0=mybir.AluOpType.add)

                        # V projection (natural layout) for this chunk
                        for tt in range(4):
                            sb_i = src_i * 16 + ch * 4 + tt
                            pv = psA.tile([128, HCOLS], f32, tag="vproj")
                            for ckt in range(8):
                                nc.tensor.matmul(
                                    pv,
                                    lhsT=xnt[:, ckt, tt * 128:(tt + 1) * 128],
                                    rhs=wv_sb[:, ckt, :],
                                    start=(ckt == 0), stop=(ckt == 7))
                            dst = v_sb[:, sb_i, :].rearrange(
                                "p (h w) -> p h w", h=HPC)[:, :, 0:HEAD_DIM]
                            nc.vector.tensor_tensor(
                                out=dst,
                                in0=pv[:].rearrange("p (h d) -> p h d", h=HPC),
                                in1=bvb[:].rearrange("p (h d) -> p h d", h=HPC),
                                op=mybir.AluOpType.add)

            # ---------------- phase B: attention ----------------
            with tc.tile_pool(name="psB", bufs=2, space="PSUM") as psB:
                for hp in range(2):
                    for tch in range(4):
                        po = [psB.tile([VW, 512], f32, tag=f"pv{h2}",
                                       name=f"po{h2}") for h2 in range(2)]
                        for sb_i in range(NSB):
                            e_t = []
                            for h2 in range(2):
                                ps = psB.tile([128, 512], f32, tag=f"sc{h2}")
                                nc.tensor.matmul(
                                    ps,
                                    lhsT=kt_sb[h2 * 64:(h2 + 1) * 64, hp,
                                               sb_i * 128:(sb_i + 1) * 128],
                                    rhs=qt_sb[h2 * 64:(h2 + 1) * 64, hp,
                                              tch * 512:(tch + 1) * 512],
                                    start=True, stop=True)
                                et = ep.tile([128, 512], bf16, tag=f"e{h2}")
                                nc.scalar.activation(
                                    out=et, in_=ps,
                                    func=mybir.ActivationFunctionType.Exp)
                                e_t.append(et)
                            for h2 in range(2):
                                h = hp * 2 + h2
                                nc.tensor.matmul(
                                    po[h2],
                                    lhsT=v_sb[:, sb_i, h * VW:(h + 1) * VW],
                                    rhs=e_t[h2],
                                    start=(sb_i == 0), stop=(sb_i == NSB - 1))
                        for h2 in range(2):
                            u = hp * 8 + tch * 2 + h2
                            zi = zp.tile([1, 512], f32, tag="zi")
                            nc.vector.reciprocal(out=zi, in_=po[h2][HEAD_DIM:VW, :])
                            nc.sync.dma_start(out=zscr[u:u + 1, :], in_=zi)
                            zb = zp.tile([64, 512], f32, tag="zb")
                            row = zscr[u:u + 1, :]
                            nc.sync.dma_start(out=zb, in_=bass.AP(
                                tensor=row.tensor, offset=row.offset,
                                ap=[[0, 64]] + list(row.ap[1:])))
                            nc.vector.tensor_tensor(
                                out=aot_sb[h2 * 64:(h2 + 1) * 64, hp,
                                           tch * 512:(tch + 1) * 512],
                                in0=po[h2][0:HEAD_DIM, :], in1=zb,
                                op=mybir.AluOpType.mult)

            # ---------------- phase C: out projection + RS ----------------
            with tc.tile_pool(name="psC", bufs=2, space="PSUM") as psC:
                for tt in range(NT):
                    for half in range(2):
                        pp = psC.tile([128, 512], f32, tag="op")
                        for kt_o in range(2):
                            nc.tensor.matmul(
                                pp,
                                lhsT=aot_sb[:, kt_o, tt * 128:(tt + 1) * 128],
                                rhs=wo_sb[:, kt_o, half * 512:(half + 1) * 512],
                                start=(kt_o == 0), stop=(kt_o == 1))
                        op_sb = st.tile([128, 512], f32, tag="opsb")
                        nc.vector.tensor_copy(op_sb, pp)
                        nc.sync.dma_start(
                            out=partial[tt * 128:(tt + 1) * 128,
                                        half * 512:(half + 1) * 512],
                            in_=op_sb)

                nc.gpsimd.collective_compute(
                    "ReduceScatter", mybir.AluOpType.add,
                    replica_groups=[[0, 1, 2, 3], [4, 5, 6, 7]],
                    ins=[partial[:]], outs=[rs_out[:]])

                for i in range(4):
                    rs_sb = st.tile([128, DIM], f32, tag="rs")
                    re_sb = st.tile([128, DIM], f32, tag="re")
                    nc.sync.dma_start(out=rs_sb, in_=rs_out[i * 128:(i + 1) * 128, :])
                    nc.sync.dma_start(out=re_sb, in_=RES[i * 128:(i + 1) * 128, :])
                    o_sb = st.tile([128, DIM], f32, tag="o")
                    nc.vector.tensor_tensor(out=o_sb, in0=rs_sb, in1=re_sb,
                                            op=mybir.AluOpType.add)
                    nc.sync.dma_start(out=OUT[i * 128:(i + 1) * 128, :], in_=o_sb)

    nc.compile()
    return nc


_NC = None


def _get_nc():
    global _NC
    if _NC is None:
        _NC = _build()
    return _NC


def kernel(x, context, w_qkv, b_qkv, w_out, b_out, ln_g, ln_b, _trace=False):
    x = np.asarray(x, np.float32)
    context = np.asarray(context, np.float32)
    w_qkv = np.asarray(w_qkv, np.float32)
    b_qkv = np.asarray(b_qkv, np.float32)
    w_out = np.asarray(w_out, np.float32)
    b_out = np.asarray(b_out, np.float32)
    ln_g = np.asarray(ln_g, np.float32)
    ln_b = np.asarray(ln_b, np.float32)

    scale = np.float32(HEAD_DIM ** -0.5)
    gw = ln_g[:, None] * w_qkv          # fold LN gamma into W
    bias_full = b_qkv + ln_b @ w_qkv    # fold LN beta into bias
    idn = np.eye(128, dtype=np.float32).astype(ml_dtypes.bfloat16)

    in_maps = []
    for c in range(N_CORES):
        b, hg = divmod(c, 4)
        qc = slice(hg * HCOLS, (hg + 1) * HCOLS)
        kc = slice(DIM + hg * HCOLS, DIM + (hg + 1) * HCOLS)
        vc = slice(2 * DIM + hg * HCOLS, 2 * DIM + (hg + 1) * HCOLS)
        in_maps.append({
            "xb": x[b], "cb": context[b],
            "wq": (gw[:, qc] * scale).astype(ml_dtypes.bfloat16),
            "wk": gw[:, kc].astype(ml_dtypes.bfloat16),
            "wv": gw[:, vc].astype(ml_dtypes.bfloat16),
            "wo": w_out[hg * HCOLS:(hg + 1) * HCOLS, :].astype(ml_dtypes.bfloat16),
            "bq": (bias_full[qc] * scale).astype(np.float32),
            "bk": bias_full[kc].astype(np.float32),
            "bv": bias_full[vc].astype(np.float32),
            "res": x[b, hg * 512:(hg + 1) * 512, :] + b_out,
            "idn": idn,
        })

    res = run_bass_kernel_spmd(_get_nc(), in_maps, CORE_IDS, trace=_trace)
    out = np.empty((B, T, DIM), np.float32)
    for c in range(N_CORES):
        b, hg = divmod(c, 4)
        out[b, hg * 512:(hg + 1) * 512, :] = res.results[c]["out"]
    if _trace:
        return out, res
    return out


# revision 5
# speedup vs baseline: 47.4292x; 47.4292x over previous
"""CrossModalityAttention Trainium2 kernel (8 NeuronCores, SPMD).

Sharding: core c -> batch b = c//4, head-group hg = c%4 (4 of 16 heads).
Each core computes LN + QKV projections for its heads, full cross-attention
(self K/V concat context K/V), and a partial output projection; partials are
ReduceScattered over the 4 cores of each batch (core gets row-quarter hg),
residual (+ b_out) added, and the host reassembles the [2, 2048, 1024] output.

Precision: LN stats/apply and softmax denominators in fp32; all matmul
operands bf16 with fp32 PSUM accumulation. Softmax skips max-subtraction
(logits are O(3) for this problem family) so only exp + ones-row-matmul
normalization is needed.
"""
import sys
import numpy as np
import ml_dtypes

for p in ("/root/.axon_site", "/root/.axon_site/_ro/trn_rl_repo",
          "/root/.axon_site/_ro/pypackages", "/opt/trn_rl_repo"):
    if p not in sys.path:
        sys.path.append(p)

import concourse.bass as bass
from concourse import bacc
import concourse.mybir as mybir
import concourse.tile as tile
from concourse.bass_utils import run_bass_kernel_spmd

f32 = mybir.dt.float32
bf16 = mybir.dt.bfloat16

B, T, S, DIM = 2, 2048, 2048, 1024
HEADS, HEAD_DIM = 16, 64
HPC = 4                   # heads per core
HCOLS = HPC * HEAD_DIM    # 256 channel columns per core
N_CORES = 8
CORE_IDS = list(range(N_CORES))
EPS = 1e-5

NT = T // 128             # 16 t-tiles per batch
NCHUNK = 4                # t-chunks of 512
NSB = (T + S) // 128      # 32 s-blocks of the concat sequence
VW = HEAD_DIM + 1         # V columns + ones column per head


def _build(trace_label=""):
    nc = bacc.Bacc("TRN2", target_bir_lowering=False, debug=False,
                   num_devices=N_CORES)

    XB = nc.dram_tensor("xb", [T, DIM], f32, kind="ExternalInput").ap()
    CB = nc.dram_tensor("cb", [S, DIM], f32, kind="ExternalInput").ap()
    WQ = nc.dram_tensor("wq", [DIM, HCOLS], bf16, kind="ExternalInput").ap()
    WK = nc.dram_tensor("wk", [DIM, HCOLS], bf16, kind="ExternalInput").ap()
    WV = nc.dram_tensor("wv", [DIM, HCOLS], bf16, kind="ExternalInput").ap()
    WO = nc.dram_tensor("wo", [HCOLS, DIM], bf16, kind="ExternalInput").ap()
    BQ = nc.dram_tensor("bq", [HCOLS], f32, kind="ExternalInput").ap()
    BK = nc.dram_tensor("bk", [HCOLS], f32, kind="ExternalInput").ap()
    BV = nc.dram_tensor("bv", [HCOLS], f32, kind="ExternalInput").ap()
    RES = nc.dram_tensor("res", [T // 4, DIM], f32, kind="ExternalInput").ap()
    IDN = nc.dram_tensor("idn", [128, 128], bf16, kind="ExternalInput").ap()

    OUT = nc.dram_tensor("out", [T // 4, DIM], f32, kind="ExternalOutput").ap()

    partial = nc.dram_tensor("partial", [T, DIM], f32).ap()
    rs_out = nc.dram_tensor("rs_out", [T // 4, DIM], f32).ap()
    zscr = nc.dram_tensor("zscr", [16, 512], f32).ap()

    with tile.TileContext(nc) as tc:
        with (
            tc.tile_pool(name="persist", bufs=1) as per,
            tc.tile_pool(name="stream", bufs=3) as st,
            tc.tile_pool(name="xnp", bufs=10) as xnp,
            tc.tile_pool(name="xntp", bufs=2) as xntp,
            tc.tile_pool(name="ep", bufs=4) as ep,
            tc.tile_pool(name="zp", bufs=4) as zp,
        ):
            # ---------------- persistent tiles ----------------
            wq_sb = per.tile([128, 8, HCOLS], bf16, tag="wq")
            wk_sb = per.tile([128, 8, HCOLS], bf16, tag="wk")
            wv_sb = per.tile([128, 8, HCOLS], bf16, tag="wv")
            wo_sb = per.tile([128, 2, DIM], bf16, tag="wo")
            nc.sync.dma_start(out=wq_sb, in_=WQ.rearrange("(a p) c -> p a c", p=128))
            nc.sync.dma_start(out=wk_sb, in_=WK.rearrange("(a p) c -> p a c", p=128))
            nc.sync.dma_start(out=wv_sb, in_=WV.rearrange("(a p) c -> p a c", p=128))
            nc.sync.dma_start(out=wo_sb, in_=WO.rearrange("(a p) c -> p a c", p=128))

            bq_sb = per.tile([128, 2], f32, tag="bq")
            bk_sb = per.tile([128, 2], f32, tag="bk")
            nc.sync.dma_start(out=bq_sb, in_=BQ.rearrange("(a p) -> p a", p=128))
            nc.sync.dma_start(out=bk_sb, in_=BK.rearrange("(a p) -> p a", p=128))
            bvb = per.tile([128, HCOLS], f32, tag="bvb")
            nc.sync.dma_start(out=bvb, in_=bass.AP(
                tensor=BV.tensor, offset=0, ap=[[0, 128], [1, HCOLS]]))

            ident = per.tile([128, 128], bf16, tag="ident")
            nc.sync.dma_start(out=ident, in_=IDN)
            eps_sb = per.tile([128, 1], f32, tag="eps")
            nc.vector.memset(eps_sb, EPS)

            qt_sb = per.tile([128, 2, T], bf16, tag="qt")      # Q^T
            kt_sb = per.tile([128, 2, T + S], bf16, tag="kt")  # K^T (concat)
            v_sb = per.tile([128, NSB, HPC * VW], bf16, tag="v")   # V | ones
            aot_sb = per.tile([128, 2, T], bf16, tag="aot")    # attn out^T

            for h in range(HPC):  # ones columns for Z rows
                nc.vector.memset(v_sb[:, :, h * VW + HEAD_DIM: (h + 1) * VW], 1.0)

            # ---------------- phase A: LN + transposes + QKV ----------------
            with tc.tile_pool(name="psA", bufs=2, space="PSUM") as psA:
                for src_i, SRC in ((0, XB), (1, CB)):
                    for ch in range(NCHUNK):
                        xn_tiles = []
                        for tt in range(4):
                            r0 = (ch * 4 + tt) * 128
                            xt = st.tile([128, DIM], f32, tag="xt")
                            nc.sync.dma_start(out=xt, in_=SRC[r0:r0 + 128, :])
                            # LN stats
                            stats = st.tile([128, 2, 6], f32, tag="stats")
                            xr = xt[:].rearrange("p (a q) -> p a q", a=2)
                            for sg in range(2):
                                nc.vector.bn_stats(out=stats[:, sg, :], in_=xr[:, sg, :])
                            mv = st.tile([128, 2], f32, tag="mv")
                            nc.vector.bn_aggr(out=mv, in_=stats)
                            rstd = st.tile([128, 1], f32, tag="rstd")
                            nc.scalar.activation(
                                out=rstd, in_=mv[:, 1:2],
                                func=mybir.ActivationFunctionType.Sqrt,
                                bias=eps_sb, scale=1.0)
                            nc.vector.reciprocal(out=rstd, in_=rstd)
                            xn = xnp.tile([128, DIM], bf16, tag="xn")
                            nc.vector.tensor_scalar(
                                out=xn, in0=xt, scalar1=mv[:, 0:1], scalar2=rstd,
                                op0=mybir.AluOpType.subtract,
                                op1=mybir.AluOpType.mult)
                            xn_tiles.append(xn)

                        # transpose chunk -> xnT [128c, 8ckt, 512t]
                        xnt = xntp.tile([128, 8, 512], bf16, tag="xnt")
                        for ckt in range(8):
                            pt = psA.tile([128, 512], bf16, tag="tp")
                            for tt in range(4):
                                nc.tensor.transpose(
                                    pt[:, tt * 128:(tt + 1) * 128],
                                    xn_tiles[tt][:, ckt * 128:(ckt + 1) * 128],
                                    ident)
                            nc.vector.tensor_copy(xnt[:, ckt, :], pt)

                        # Q^T / K^T projections for this chunk
                        wlist = ([(wq_sb, bq_sb, qt_sb, 0), (wk_sb, bk_sb, kt_sb, 0)]
                                 if src_i == 0 else [(wk_sb, bk_sb, kt_sb, T)])
                        for (w, bia, dst, off) in wlist:
                            for kt_o in range(2):
                                pq = psA.tile([128, 512], f32, tag="proj")
                                for ckt in range(8):
                                    nc.tensor.matmul(
                                        pq,
                                        lhsT=w[:, ckt, kt_o * 128:(kt_o + 1) * 128],
                                        rhs=xnt[:, ckt, :],
                                        start=(ckt == 0), stop=(ckt == 7))
                                nc.vector.tensor_scalar(
                                    out=dst[:, kt_o, off + ch * 512: off + (ch + 1) * 512],
                                    in0=pq, scalar1=bia[:, kt_o:kt_o + 1],
                                    scalar2=None, op0=mybir.AluOpType.add)

                        # V projection (natural layout) for this chunk
                        for tt in range(4):
                            sb_i = src_i * 16 + ch * 4 + tt
                            pv = psA.tile([128, HCOLS], f32, tag="vproj")
                            for ckt in range(8):
                                nc.tensor.matmul(
                                    pv,
                                    lhsT=xnt[:, ckt, tt * 128:(tt + 1) * 128],
                                    rhs=wv_sb[:, ckt, :],
                                    start=(ckt == 0), stop=(ckt == 7))
                            dst = v_sb[:, sb_i, :].rearrange(
                                "p (h w) -> p h w", h=HPC)[:, :, 0:HEAD_DIM]
                            nc.vector.tensor_tensor(
                                out=dst,
                                in0=pv[:].rearrange("p (h d) -> p h d", h=HPC),
                                in1=bvb[:].rearrange("p (h d) -> p h d", h=HPC),
                                op=mybir.AluOpType.add)

            # ---------------- phase B: attention ----------------
            with tc.tile_pool(name="psB", bufs=2, space="PSUM") as psB:
                for hp in range(2):
                    for tch in range(4):
                        po = [psB.tile([VW, 512], f32, tag=f"pv{h2}",
                                       name=f"po{h2}") for h2 in range(2)]
                        for sb_i in range(NSB):
                            e_t = []
                            for h2 in range(2):
                                ps = psB.tile([128, 512], f32, tag=f"sc{h2}")
                                nc.tensor.matmul(
                                    ps,
                                    lhsT=kt_sb[h2 * 64:(h2 + 1) * 64, hp,
                                               sb_i * 128:(sb_i + 1) * 128],
                                    rhs=qt_sb[h2 * 64:(h2 + 1) * 64, hp,
                                              tch * 512:(tch + 1) * 512],
                                    start=True, stop=True)
                                et = ep.tile([128, 512], bf16, tag=f"e{h2}")
                                nc.scalar.activation(
                                    out=et, in_=ps,
                                    func=mybir.ActivationFunctionType.Exp)
                                e_t.append(et)
                            for h2 in range(2):
                                h = hp * 2 + h2
                                nc.tensor.matmul(
                                    po[h2],
                                    lhsT=v_sb[:, sb_i, h * VW:(h + 1) * VW],
                                    rhs=e_t[h2],
                                    start=(sb_i == 0), stop=(sb_i == NSB - 1))
                        for h2 in range(2):
                            u = hp * 8 + tch * 2 + h2
                            zi = zp.tile([1, 512], f32, tag="zi")
                            nc.vector.reciprocal(out=zi, in_=po[h2][HEAD_DIM:VW, :])
                            nc.sync.dma_start(out=zscr[u:u + 1, :], in_=zi)
                            zb = zp.tile([64, 512], f32, tag="zb")
                            row = zscr[u:u + 1, :]
                            nc.sync.dma_start(out=zb, in_=bass.AP(
                                tensor=row.tensor, offset=row.offset,
                                ap=[[0, 64]] + list(row.ap[1:])))
                            nc.vector.tensor_tensor(
                                out=aot_sb[h2 * 64:(h2 + 1) * 64, hp,
                                           tch * 512:(tch + 1) * 512],
                                in0=po[h2][0:HEAD_DIM, :], in1=zb,
                                op=mybir.AluOpType.mult)

            # ---------------- phase C: out projection + RS ----------------
            with tc.tile_pool(name="psC", bufs=2, space="PSUM") as psC:
                for tt in range(NT):
                    for half in range(2):
                        pp = psC.tile([128, 512], f32, tag="op")
                        for kt_o in range(2):
                            nc.tensor.matmul(
                                pp,
                                lhsT=aot_sb[:, kt_o, tt * 128:(tt + 1) * 128],
                                rhs=wo_sb[:, kt_o, half * 512:(half + 1) * 512],
                                start=(kt_o == 0), stop=(kt_o == 1))
                        op_sb = st.tile([128, 512], f32, tag="opsb")
                        nc.vector.tensor_copy(op_sb, pp)
                        nc.sync.dma_start(
                            out=partial[tt * 128:(tt + 1) * 128,
                                        half * 512:(half + 1) * 512],
                            in_=op_sb)

                nc.gpsimd.collective_compute(
                    "ReduceScatter", mybir.AluOpType.add,
                    replica_groups=[[0, 1, 2, 3], [4, 5, 6, 7]],
                    ins=[partial[:]], outs=[rs_out[:]])

                for i in range(4):
                    rs_sb = st.tile([128, DIM], f32, tag="rs")
                    re_sb = st.tile([128, DIM], f32, tag="re")
                    nc.sync.dma_start(out=rs_sb, in_=rs_out[i * 128:(i + 1) * 128, :])
                    nc.sync.dma_start(out=re_sb, in_=RES[i * 128:(i + 1) * 128, :])
                    o_sb = st.tile([128, DIM], f32, tag="o")
                    nc.vector.tensor_tensor(out=o_sb, in0=rs_sb, in1=re_sb,
                                            op=mybir.AluOpType.add)
                    nc.sync.dma_start(out=OUT[i * 128:(i + 1) * 128, :], in_=o_sb)

    nc.compile()
    return nc


_NC = None


def _get_nc():
    global _NC
    if _NC is None:
        _NC = _build()
    return _NC


def make_in_maps(x, context, w_qkv, b_qkv, w_out, b_out, ln_g, ln_b):
    x = np.asarray(x, np.float32)
    context = np.asarray(context, np.float32)
    w_qkv = np.asarray(w_qkv, np.float32)
    b_qkv = np.asarray(b_qkv, np.float32)
    w_out = np.asarray(w_out, np.float32)
    b_out = np.asarray(b_out, np.float32)
    ln_g = np.asarray(ln_g, np.float32)
    ln_b = np.asarray(ln_b, np.float32)

    scale = np.float32(HEAD_DIM ** -0.5)
    gw = ln_g[:, None] * w_qkv          # fold LN gamma into W
    bias_full = b_qkv + ln_b @ w_qkv    # fold LN beta into bias
    idn = np.eye(128, dtype=np.float32).astype(ml_dtypes.bfloat16)

    in_maps = []
    for c in range(N_CORES):
        b, hg = divmod(c, 4)
        qc = slice(hg * HCOLS, (hg + 1) * HCOLS)
        kc = slice(DIM + hg * HCOLS, DIM + (hg + 1) * HCOLS)
        vc = slice(2 * DIM + hg * HCOLS, 2 * DIM + (hg + 1) * HCOLS)
        in_maps.append({
            "xb": x[b], "cb": context[b],
            "wq": (gw[:, qc] * scale).astype(ml_dtypes.bfloat16),
            "wk": gw[:, kc].astype(ml_dtypes.bfloat16),
            "wv": gw[:, vc].astype(ml_dtypes.bfloat16),
            "wo": w_out[hg * HCOLS:(hg + 1) * HCOLS, :].astype(ml_dtypes.bfloat16),
            "bq": (bias_full[qc] * scale).astype(np.float32),
            "bk": bias_full[kc].astype(np.float32),
            "bv": bias_full[vc].astype(np.float32),
            "res": x[b, hg * 512:(hg + 1) * 512, :] + b_out,
            "idn": idn,
        })
    return in_maps


def kernel(x, context, w_qkv, b_qkv, w_out, b_out, ln_g, ln_b):
    in_maps = make_in_maps(x, context, w_qkv, b_qkv, w_out, b_out, ln_g, ln_b)
    res = run_bass_kernel_spmd(_get_nc(), in_maps, CORE_IDS)
    out = np.empty((B, T, DIM), np.float32)
    for c in range(N_CORES):
        b, hg = divmod(c, 4)
        out[b, hg * 512:(hg + 1) * 512, :] = res.results[c]["out"]
    return out
